# revision 39
# baseline (speedup 1.0000x reference)
"""GATv2 2-layer GNN (nn_ActorNetwork) on 8 TRN2 NeuronCores.

Strategy (v2):
- Host: add self-loops, sort edges by dst, shard nodes (and their incoming
  edges) across 8 cores in contiguous 2500-node ranges so segment softmax is
  core-local. Within each 128-dst chunk, edges are sorted by src. Channels
  permuted (positive-att first); att folded into the weight columns
  (signed, like table = x @ (W * att)); the Wr-side weights carry an extra
  sum-column (Wr@1) so the linear part of the attention score rides the
  node GEMM and the expansion matmul.
- Device per core: replicated layer-1 GEMM (x replicated); dma_gather of
  256B-aligned table rows per edge (src); xr[dst] expansion as fp8 mask
  matmul PLUS an identity-matmul accumulate of the gathered rows, so
  v = xl[src]+xr[dst] is formed on the TensorEngine in PSUM; Act copies v
  to SBUF bf16 in 2-tile groups; score e = 0.4*(1.5*(gsum+rsum) +
  sum_pos|v| - sum_neg|v|) where the |v| sums and the gathered-row sum come
  from fused DVE tensor_scalar+accum_out ops (4x mode); exp via Act
  (scale=0.4); the dst-scatter mask is generated on-chip fused with the ee
  scale ((iota==dm)*ee) and the aggregation runs as one matmul per tile
  with a ones-column providing the softmax denominator. PSUM->SBUF table
  copies ride the idle Pool engine. Layer 2 is data-parallel over nodes
  with one AllGather of the layer-2 table (quarters, overlapped with
  layer-1 edge compute). No Prelu anywhere so a single activation-table
  set (exp/ln/identity/abs) is loaded once.
"""
import os
import numpy as np
import ml_dtypes

NO_CC = os.environ.get("K_NO_CC", "0") == "1"  # replace collectives (sim only)

N, E0, D, H, A = 20000, 320000, 256, 256, 128
NC = 8
NPC = N // NC            # 2500 real nodes per core
NCHUNK = 20              # chunks of 128 dst nodes
NPCP = NCHUNK * 128      # 2560 padded nodes per core
NN = NC * NPCP           # padded global node rows (20480)
NEG_SLOPE = 0.2
EPS = 1e-30
PSZ = [6, 6, 6, 2]       # layer-2 table piece sizes (chunks)
PST = [0, 6, 12, 18]     # piece starts

_CACHE = {}


def _preprocess(x, edge_index, Wl1, Wr1, att1, b1, Wl2, Wr2, att2, b2):
    bf16 = ml_dtypes.bfloat16
    fp8 = ml_dtypes.float8_e4m3

    perm1 = np.argsort(att1 < 0, kind="stable")
    perm2 = np.argsort(att2 < 0, kind="stable")
    P1p = int((att1 >= 0).sum())
    P2p = int((att2 >= 0).sum())
    a1p = att1[perm1]
    a2p = att2[perm2]

    # signed fold: table entries are att_c * (W x)_c; the score's linear
    # part is then a plain row-sum, and |entry| = |att_c (Wx)_c| for the
    # abs part (signs handled by the pos/neg column split). The Wr-side
    # linear term is constant per dst segment, so it cancels in the
    # softmax and is dropped entirely.
    W1 = np.concatenate([(Wl1 * att1[None, :])[:, perm1],
                         (Wr1 * att1[None, :])[:, perm1]], axis=1).astype(bf16)
    Wl2p = (Wl2 * att2[None, :])[perm1][:, perm2]
    Wr2p = (Wr2 * att2[None, :])[perm1][:, perm2]
    W2 = np.concatenate([Wl2p, Wr2p], axis=1).astype(bf16)

    inv1 = (1.0 / a1p).astype(np.float32).reshape(2, 128).T.copy()  # [128, 2]
    b1c = b1[perm1].astype(np.float32).reshape(2, 128).T.copy()
    inv2_mat = np.tile((1.0 / a2p).astype(np.float32)[None, :], (128, 1))
    b2_mat = np.tile(b2[perm2].astype(np.float32)[None, :], (128, 1))

    # edges + self loops, sharded by dst range, chunked by dst>>7
    loops = np.arange(N, dtype=np.int64)
    src = np.concatenate([edge_index[0].astype(np.int64), loops])
    dst = np.concatenate([edge_index[1].astype(np.int64), loops])
    order = np.argsort(dst, kind="stable")
    src, dst = src[order], dst[order]
    core_of = dst // NPC
    dstl = dst - core_of * NPC
    chunk = dstl >> 7
    counts = np.zeros((NC, NCHUNK), np.int64)
    np.add.at(counts, (core_of, chunk), 1)
    TC = np.ceil(counts.max(axis=0) / 128).astype(np.int64)
    EC = TC * 128
    NI = int(EC.sum())
    T_total = int(TC.sum())

    pad = (src // NPC) * NPCP + (src % NPC)       # padded node id
    # layer1 table rows: partition-major flat [128, NN/128, 256]; layer2
    # table in 4 uneven pieces of [6,6,6,2] chunks (small tail so the last
    # AllGather barely delays the layer-2 edge phase)
    r1 = (pad % 128) * (NN // 128) + pad // 128
    jj = (pad % NPCP) // 128          # chunk index of the src node
    qq = np.minimum(jj // 6, 3)       # piece
    psz = np.array(PSZ)[qq]
    pst = np.array(PST)[qq]
    r2 = pst * (NC * 128) + ((pad // NPCP) * 128 + pad % NPCP % 128) * psz \
        + (jj - pst)
    src_row = r1.astype(np.int32)
    src_row2 = r2.astype(np.int32)

    per_core = []
    for c in range(NC):
        m = core_of == c
        s_c, s2_c, dl_c, ch_c = src_row[m], src_row2[m], dstl[m], chunk[m]
        sr = np.zeros(NI, np.int32)       # layer-1 table rows (pad: 0)
        sr2 = np.zeros(NI, np.int32)      # layer-2 table rows (pad: 0)
        dm = np.full(NI, 999, np.int32)   # dst-local-in-chunk (pad: no match)
        off = 0
        for q in range(NCHUNK):
            sel = ch_c == q
            n = int(sel.sum())
            sq, s2q, dq = s_c[sel], s2_c[sel], dl_c[sel] - q * 128
            o2 = np.argsort(sq, kind="stable")  # src order -> HBM locality
            sr[off:off + n] = sq[o2]
            sr2[off:off + n] = s2q[o2]
            dm[off:off + n] = dq[o2]
            off += int(EC[q])

        def wrap(a):
            out = np.zeros((128, NI // 16), np.int16)
            off_e = 0
            for q in range(NCHUNK):
                e = int(EC[q])
                blk = a[off_e:off_e + e].reshape(e // 16, 16).T.astype(np.int16)
                out[:, off_e // 16:(off_e + e) // 16] = np.tile(blk, (8, 1))
                off_e += e
            return out
        idx_src = wrap(sr)
        idx_src2 = wrap(sr2)
        dmt = dm.reshape(T_total, 128)
        dmf = np.ascontiguousarray(dmt.T).astype(np.float32)  # [128, T_total]
        masks = (dmt[:, :, None] == np.arange(128)[None, None, :])
        masksDE = np.ascontiguousarray(
            masks.transpose(2, 0, 1).reshape(128, T_total * 128)
        ).astype(np.float32).astype(fp8)          # [d-part, (t, e)]

        xoT = np.zeros((D, NPCP), np.float32)
        xoT[:, :NPC] = x[c * NPC:(c + 1) * NPC].T
        per_core.append(dict(
            xoT=xoT.astype(bf16), W1=W1, W2=W2,
            inv1=inv1, b1c=b1c, inv2_mat=inv2_mat, b2_mat=b2_mat,
            idx_src=idx_src, idx_src2=idx_src2, dmf=dmf,
            masksDE=masksDE,
            ident=np.eye(128, dtype=bf16),
            iota=np.tile(np.arange(128, dtype=np.float32)[None, :],
                         (128, 1)).astype(bf16),
        ))

    # replicated padded x^T (same for all cores)
    xf = np.zeros((NN, D), np.float32)
    for c in range(NC):
        xf[c * NPCP:c * NPCP + NPC] = x[c * NPC:(c + 1) * NPC]
    xfT = np.ascontiguousarray(xf.T).astype(bf16)
    for c in range(NC):
        per_core[c]["xfT"] = xfT

    return per_core, [int(t) for t in TC], P1p, P2p, NI, perm2


def _build(TC, P1p, P2p, NI):
    from concourse import mybir, tile, bacc

    F32 = mybir.dt.float32
    BF16 = mybir.dt.bfloat16
    FP8 = mybir.dt.float8e4
    I16 = mybir.dt.int16
    AF = mybir.ActivationFunctionType
    OP = mybir.AluOpType
    T_total = sum(TC)
    TCm = max(TC)

    nc = bacc.Bacc("TRN2", target_bir_lowering=False, debug=False,
                   num_devices=NC)
    xfT_d = nc.dram_tensor("xfT", [D, NN], BF16, kind="ExternalInput")
    xoT_d = nc.dram_tensor("xoT", [D, NPCP], BF16, kind="ExternalInput")
    W1_d = nc.dram_tensor("W1", [D, 2 * H], BF16, kind="ExternalInput")
    W2_d = nc.dram_tensor("W2", [H, 2 * A], BF16, kind="ExternalInput")
    inv1_d = nc.dram_tensor("inv1", [128, 2], F32, kind="ExternalInput")
    b1c_d = nc.dram_tensor("b1c", [128, 2], F32, kind="ExternalInput")
    inv2_d = nc.dram_tensor("inv2_mat", [128, A], F32, kind="ExternalInput")
    b2m_d = nc.dram_tensor("b2_mat", [128, A], F32, kind="ExternalInput")
    isrc_d = nc.dram_tensor("idx_src", [128, NI // 16], I16, kind="ExternalInput")
    isrc2_d = nc.dram_tensor("idx_src2", [128, NI // 16], I16, kind="ExternalInput")
    dmf_d = nc.dram_tensor("dmf", [128, T_total], F32, kind="ExternalInput")
    mDE_d = nc.dram_tensor("masksDE", [128, T_total * 128], FP8, kind="ExternalInput")
    iden_d = nc.dram_tensor("ident", [128, 128], BF16, kind="ExternalInput")
    iota_d = nc.dram_tensor("iota", [128, 128], BF16, kind="ExternalInput")
    out_d = nc.dram_tensor("out", [NPCP, A], F32, kind="ExternalOutput")

    with tile.TileContext(nc) as tc:
        with tc.tile_pool(name="const", bufs=1) as cp, \
             tc.tile_pool(name="tabs", bufs=1) as tp, \
             tc.tile_pool(name="edge", bufs=2) as ep, \
             tc.tile_pool(name="small", bufs=2) as sp, \
             tc.tile_pool(name="psg", bufs=2, space="PSUM") as psg, \
             tc.tile_pool(name="psa", bufs=2, space="PSUM") as psa, \
             tc.tile_pool(name="psv", bufs=2, space="PSUM") as psv, \
             tc.tile_pool(name="dram", bufs=1, space="DRAM") as dp:
            # ---- constants / inputs to SBUF
            xoT_sb = cp.tile([128, 2, NPCP], BF16)
            nc.sync.dma_start(out=xoT_sb[:, 0, :], in_=xoT_d[0:128, :])
            nc.sync.dma_start(out=xoT_sb[:, 1, :], in_=xoT_d[128:256, :])
            W1_sb = cp.tile([128, 2, 2 * H], BF16)
            nc.sync.dma_start(out=W1_sb[:, 0, :], in_=W1_d[0:128, :])
            nc.sync.dma_start(out=W1_sb[:, 1, :], in_=W1_d[128:256, :])
            W2_sb = cp.tile([128, 2, 2 * A], BF16)
            nc.sync.dma_start(out=W2_sb[:, 0, :], in_=W2_d[0:128, :])
            nc.sync.dma_start(out=W2_sb[:, 1, :], in_=W2_d[128:256, :])
            inv1_sb = cp.tile([128, 2], F32)
            nc.sync.dma_start(out=inv1_sb[:], in_=inv1_d[:])
            b1c_sb = cp.tile([128, 2], F32)
            nc.sync.dma_start(out=b1c_sb[:], in_=b1c_d[:])
            inv2_sb = cp.tile([128, A], F32)
            nc.sync.dma_start(out=inv2_sb[:], in_=inv2_d[:])
            b2m_sb = cp.tile([128, A], F32)
            nc.sync.dma_start(out=b2m_sb[:], in_=b2m_d[:])
            iden_sb = cp.tile([128, 128], BF16)
            nc.sync.dma_start(out=iden_sb[:], in_=iden_d[:])
            iota_sb = cp.tile([128, 128], BF16)
            nc.sync.dma_start(out=iota_sb[:], in_=iota_d[:])
            ones_sb = cp.tile([128, 1], BF16)
            nc.vector.memset(ones_sb[:], 1.0)
            # the big replicated-x load comes after the GEMM constants so
            # the Wr-side GEMMs can start immediately; edge-phase metadata
            # (indices, dm) loads after it since gathers only start once
            # the table is written anyway
            xfT_sb = cp.tile([128, 2, NN], BF16, tag="xfT")
            for grp in range(NC):
                sl = slice(grp * NPCP, (grp + 1) * NPCP)
                nc.sync.dma_start(out=xfT_sb[:, 0, sl], in_=xfT_d[0:128, sl])
                nc.sync.dma_start(out=xfT_sb[:, 1, sl], in_=xfT_d[128:256, sl])
            isrc_sb = cp.tile([128, NI // 16], I16)
            nc.sync.dma_start(out=isrc_sb[:], in_=isrc_d[:])
            isrc2_sb = cp.tile([128, NI // 16], I16)
            nc.sync.dma_start(out=isrc2_sb[:], in_=isrc2_d[:])
            dmf_sb = cp.tile([128, T_total], F32)
            nc.sync.dma_start(out=dmf_sb[:], in_=dmf_d[:])

            t1full = dp.tile([128, NN // 128, H], BF16)
            t2full = dp.tile([NN, A], BF16)
            t2g = [dp.tile([NC * 128, PSZ[g], A], BF16,
                           **({} if NO_CC else dict(addr_space="Shared")),
                           name=f"t2g_{g}") for g in range(4)]

            # ---- layer 1 tables: replicated GEMM over all nodes (Wl side),
            # own-range GEMM for the Wr side incl. sum-column (stays in
            # SBUF). PSUM->SBUF copies ride the idle Pool engine.
            def any_copy(i, out, in_):
                # round-robin PSUM->SBUF copies across the three idle-ish
                # engines so the table phase isn't gated by one of them
                r = i % 3
                if r == 0:
                    nc.vector.tensor_copy(out=out, in_=in_)
                elif r == 1:
                    nc.scalar.activation(out=out, in_=in_, func=AF.Identity)
                else:
                    nc.gpsimd.tensor_copy(out=out, in_=in_)

            T1r = tp.tile([128, NCHUNK, H], BF16, tag="tr")
            for m in range(NCHUNK):
                ps = psg.tile([128, H], F32, space="PSUM")
                for k in range(2):
                    nc.tensor.matmul(
                        out=ps[:], lhsT=xoT_sb[:, k, m * 128:(m + 1) * 128],
                        rhs=W1_sb[:, k, H:2 * H], start=(k == 0), stop=(k == 1))
                any_copy(m, T1r[:, m, :], ps[:])
            for grp in range(NC):
                grp_sb = tp.tile([128, NCHUNK, H], BF16, tag="town", bufs=2)
                for m in range(NCHUNK):
                    mg = grp * NCHUNK + m
                    ps = psg.tile([128, H], F32, space="PSUM")
                    for k in range(2):
                        nc.tensor.matmul(
                            out=ps[:],
                            lhsT=xfT_sb[:, k, mg * 128:(mg + 1) * 128],
                            rhs=W1_sb[:, k, 0:H], start=(k == 0), stop=(k == 1))
                    any_copy(mg, grp_sb[:, m, :], ps[:])
                nc.sync.dma_start(
                    out=t1full[:, grp * NCHUNK:(grp + 1) * NCHUNK, :],
                    in_=grp_sb[:])

            def edge_layer(CH, Pp, full_dram, idx_sb, r_of, out_cb):
                """CH channels; Pp positive-att channels; r_of(c) gives the
                [128, CH] Wr-side table slice; out_cb(c, out_ps, den_r)
                consumes the chunk PSUM accumulator [128, CH+1] (0:CH
                numer, CH denom)."""
                off_e = 0
                ti0 = 0
                for c in range(NCHUNK):
                    T = TC[c]
                    ECc = T * 128
                    g = ep.tile([128, TCm, CH], BF16, tag="g", bufs=3)
                    src_flat = (full_dram[:] if len(full_dram.shape) == 2
                                else full_dram[:].rearrange("p m c -> (p m) c"))
                    nc.gpsimd.dma_gather(
                        g[:, 0:T, :],
                        src_flat,
                        idx_sb[:, off_e // 16:(off_e + ECc) // 16],
                        ECc, ECc, CH, single_packet=False)
                    mkDE = ep.tile([128, TCm * 128], FP8, tag="mkDE")
                    nc.sync.dma_start(
                        out=mkDE[:, 0:ECc],
                        in_=mDE_d[:, ti0 * 128:ti0 * 128 + ECc])
                    # v[e,:] = g[e,:] + T_r[dst_e,:]: expansion matmul + an
                    # identity accumulate, grouped 2 tiles per PSUM buffer.
                    # Act's accum_out on the per-tile PSUM->SBUF copy gives
                    # sum_c v (the score's linear part: per-dst shifts
                    # cancel in the softmax).
                    eA = sp.tile([128, TCm], F32, tag="eA", bufs=3)
                    eB = sp.tile([128, TCm], F32, tag="eB", bufs=3)
                    vs = sp.tile([128, TCm], F32, tag="vs", bufs=3)
                    vsb = sp.tile([128, TCm, CH], BF16, tag="vsb", bufs=2)
                    for t0 in range(0, T, 2):
                        G = min(2, T - t0)
                        vps = psv.tile([128, 2, 256], F32, space="PSUM",
                                       bufs=3)
                        for t in range(t0, t0 + G):
                            nc.tensor.matmul(
                                out=vps[:, t - t0, 0:CH],
                                lhsT=mkDE[:, t * 128:(t + 1) * 128],
                                rhs=r_of(c), start=True, stop=False)
                            nc.tensor.matmul(
                                out=vps[:, t - t0, 0:CH],
                                lhsT=iden_sb[:],
                                rhs=g[:, t, :], start=False, stop=True)
                        for t in range(t0, t0 + G):
                            nc.scalar.activation(
                                out=vsb[:, t, :], in_=vps[:, t - t0, 0:CH],
                                func=AF.Identity, accum_out=vs[:, t:t + 1])
                        for t in range(t0, t0 + G):
                            zj = sp.tile([128, CH - Pp], BF16, tag="zj")
                            nc.vector.tensor_scalar(
                                out=zj[:, 0:Pp], in0=vsb[:, t, 0:Pp],
                                scalar1=0.0, scalar2=None, op0=OP.abs_max,
                                accum_out=eA[:, t:t + 1])
                            nc.vector.tensor_scalar(
                                out=zj[:, 0:CH - Pp], in0=vsb[:, t, Pp:CH],
                                scalar1=0.0, scalar2=None, op0=OP.abs_max,
                                accum_out=eB[:, t:t + 1])
                    # e = 0.4*(1.5*vs + eA - eB); ee = exp(e)
                    d1 = sp.tile([128, TCm], F32, tag="d1", bufs=3)
                    nc.vector.tensor_tensor(
                        out=d1[:, 0:T], in0=eA[:, 0:T], in1=eB[:, 0:T],
                        op=OP.subtract)
                    d15 = sp.tile([128, TCm], F32, tag="d15", bufs=3)
                    nc.vector.tensor_scalar_mul(
                        d15[:, 0:T], vs[:, 0:T], 1.5)
                    d2 = sp.tile([128, TCm], F32, tag="d2", bufs=3)
                    nc.vector.tensor_tensor(
                        out=d2[:, 0:T], in0=d1[:, 0:T], in1=d15[:, 0:T],
                        op=OP.add)
                    ee = sp.tile([128, TCm], F32, tag="ee", bufs=3)
                    nc.scalar.activation(out=ee[:, 0:T], in_=d2[:, 0:T],
                                         func=AF.Exp, scale=0.4)
                    # aggregation: mask rows scaled by ee; a ones-column
                    # matmul accumulates the softmax denominator
                    out_ps = psa.tile([128, CH + 1], F32, space="PSUM")
                    for t in range(T):
                        mk = sp.tile([128, 128], BF16, tag="mk", bufs=4)
                        nc.vector.tensor_scalar(
                            out=mk[:], in0=iota_sb[:],
                            scalar1=dmf_sb[:, ti0 + t:ti0 + t + 1],
                            scalar2=ee[:, t:t + 1],
                            op0=OP.is_equal, op1=OP.mult)
                        nc.tensor.matmul(
                            out=out_ps[:, 0:CH], lhsT=mk[:],
                            rhs=g[:, t, :],
                            start=(t == 0), stop=(t == T - 1))
                        nc.tensor.matmul(
                            out=out_ps[:, CH:CH + 1], lhsT=mk[:],
                            rhs=ones_sb[:],
                            start=(t == 0), stop=(t == T - 1))
                    den = sp.tile([128, 1], F32, tag="den")
                    nc.vector.tensor_scalar(
                        out=den[:], in0=out_ps[:, CH:CH + 1], scalar1=EPS,
                        scalar2=None, op0=OP.add)
                    den_r = sp.tile([128, 1], F32, tag="denr")
                    nc.vector.reciprocal(out=den_r[:], in_=den[:])
                    out_cb(c, out_ps, den_r)
                    off_e += ECc
                    ti0 += T

            # ---- layer 1 edge phase -> h^T (reuses xfT's slot: xfT is dead
            # after the L1 GEMM, and hT [128, 2, NPCP] fits in its slot)
            hT = cp.tile([128, 2, NPCP], BF16, tag="xfT")
            T2both = tp.tile([128, NCHUNK, 2 * A], BF16, tag="town", bufs=2)
            t2own_q = [dp.tile([128, PSZ[g], A], BF16, name=f"t2own_{g}")
                       for g in range(4)]

            def l1_out(c, out_ps, den_r):
                sc = sp.tile([128, H], BF16, tag="sc1")
                nc.scalar.activation(out=sc[:], in_=out_ps[:, 0:H],
                                     func=AF.Identity, scale=den_r[:])
                for b in range(2):
                    tps = psg.tile([128, 128], BF16, space="PSUM", bufs=1)
                    nc.tensor.transpose(
                        out=tps[:], in_=sc[:, b * 128:(b + 1) * 128],
                        identity=iden_sb[:])
                    u = sp.tile([128, 128], BF16, tag="u")
                    nc.scalar.activation(
                        out=u[:], in_=tps[:], func=AF.Identity,
                        scale=inv1_sb[:, b:b + 1], bias=b1c_sb[:, b:b + 1])
                    ng = sp.tile([128, 128], BF16, tag="ng")
                    nc.vector.tensor_scalar_min(ng[:], u[:], 0.0)
                    ex = sp.tile([128, 128], BF16, tag="ex")
                    nc.scalar.activation(out=ex[:], in_=ng[:], func=AF.Exp)
                    px = sp.tile([128, 128], BF16, tag="px")
                    nc.vector.tensor_scalar_max(px[:], u[:], 0.0)
                    s2 = sp.tile([128, 128], BF16, tag="s2")
                    nc.vector.tensor_tensor(out=s2[:], in0=ex[:], in1=px[:],
                                            op=OP.add)
                    nc.vector.tensor_scalar_add(
                        hT[:, b, c * 128:(c + 1) * 128], s2[:], -1.0)
                ps = psg.tile([128, 2 * A], F32, space="PSUM")
                for k in range(2):
                    nc.tensor.matmul(
                        out=ps[:], lhsT=hT[:, k, c * 128:(c + 1) * 128],
                        rhs=W2_sb[:, k, :], start=(k == 0), stop=(k == 1))
                nc.vector.tensor_copy(out=T2both[:, c, :], in_=ps[:])
                piece_end = {PST[i] + PSZ[i]: i for i in range(4)}
                if (c + 1) in piece_end:
                    g = piece_end[c + 1]
                    nc.sync.dma_start(
                        out=t2own_q[g][:],
                        in_=T2both[:, PST[g]:PST[g] + PSZ[g], 0:A])
                    if NO_CC:
                        for cc in range(NC):
                            nc.sync.dma_start(
                                out=t2g[g][cc * 128:(cc + 1) * 128],
                                in_=t2own_q[g][:])
                    else:
                        nc.gpsimd.collective_compute(
                            "AllGather", mybir.AluOpType.bypass,
                            replica_groups=[list(range(NC))],
                            ins=[t2own_q[g][:].opt()], outs=[t2g[g][:].opt()])
                    r0 = PST[g] * NC * 128
                    r1_ = r0 + PSZ[g] * NC * 128
                    nc.sync.dma_start(
                        out=t2full[r0:r1_, :],
                        in_=t2g[g][:].rearrange("r m c -> (r m) c"))

            edge_layer(H, P1p, t1full, isrc_sb,
                       lambda c: T1r[:, c, :], l1_out)

            # ---- layer 2 edge phase -> log_softmax -> out. The final
            # ln(sum exp) is batched over all chunks at the end so the Act
            # engine never swaps tables (Ln lives in a different act-table
            # set than Exp) during the pipeline.
            out_sb = tp.tile([128, NCHUNK, A], F32, tag="tr")
            rmaxs = tp.tile([128, NCHUNK], F32, tag="rmaxs")
            rsums = tp.tile([128, NCHUNK], F32, tag="rsums")

            def l2_out(c, out_ps, den_r):
                sc = sp.tile([128, A], F32, tag="sc2")
                nc.scalar.activation(out=sc[:], in_=out_ps[:, 0:A],
                                     func=AF.Identity, scale=den_r[:])
                lg = sp.tile([128, A], F32, tag="lg")
                nc.vector.tensor_tensor(out=lg[:], in0=sc[:],
                                        in1=inv2_sb[:], op=OP.mult)
                lg2 = sp.tile([128, A], F32, tag="lg2")
                nc.vector.tensor_tensor(out=lg2[:], in0=lg[:],
                                        in1=b2m_sb[:], op=OP.add)
                rmax = rmaxs[:, c:c + 1]
                nc.vector.tensor_reduce(
                    out=rmax, in_=lg2[:], axis=mybir.AxisListType.X,
                    op=OP.max)
                nrm = sp.tile([128, 1], F32, tag="nrm")
                nc.vector.tensor_scalar_mul(nrm[:], rmax, -1.0)
                ex2 = sp.tile([128, A], F32, tag="ex2")
                nc.scalar.activation(out=ex2[:], in_=lg2[:], func=AF.Exp,
                                     bias=nrm[:])
                nc.vector.tensor_reduce(
                    out=rsums[:, c:c + 1], in_=ex2[:],
                    axis=mybir.AxisListType.X, op=OP.add)
                nc.vector.tensor_copy(out=out_sb[:, c, :], in_=lg2[:])

            edge_layer(A, P2p, t2full, isrc2_sb,
                       lambda c: T2both[:, c, A:2 * A], l2_out)
            lsums = tp.tile([128, NCHUNK], F32, tag="lsums")
            nc.scalar.activation(out=lsums[:], in_=rsums[:], func=AF.Ln)
            shifts = tp.tile([128, NCHUNK], F32, tag="shifts")
            nc.vector.tensor_tensor(out=shifts[:], in0=rmaxs[:],
                                    in1=lsums[:], op=OP.add)
            for c in range(NCHUNK):
                nc.vector.tensor_scalar(
                    out=out_sb[:, c, :], in0=out_sb[:, c, :],
                    scalar1=shifts[:, c:c + 1],
                    scalar2=None, op0=OP.subtract)
            nc.sync.dma_start(
                out=out_d[:].rearrange("(m p) c -> p m c", p=128),
                in_=out_sb[:])

    nc.compile()
    return nc


def kernel(**inputs):
    from concourse.bass_utils import run_bass_kernel_spmd

    per_core, TC, P1p, P2p, NI, perm2 = _preprocess(
        inputs["x"], inputs["edge_index"], inputs["Wl1"], inputs["Wr1"],
        inputs["att1"], inputs["b1"], inputs["Wl2"], inputs["Wr2"],
        inputs["att2"], inputs["b2"])

    key = (tuple(TC), P1p, P2p, NI)
    if key not in _CACHE:
        _CACHE[key] = _build(TC, P1p, P2p, NI)
    nc = _CACHE[key]

    res = run_bass_kernel_spmd(nc, per_core, list(range(NC)))
    global LAST_RESULT
    LAST_RESULT = res
    out = np.empty((N, A), np.float32)
    for c in range(NC):
        dev = res.results[c]["out"]  # [NPCP, A] in perm2 channel space
        out[c * NPC:(c + 1) * NPC, perm2] = dev[:NPC]
    return out


# revision 45
# speedup vs baseline: 1.0742x; 1.0742x over previous
"""GATv2 2-layer GNN (nn_ActorNetwork) on 8 TRN2 NeuronCores.

Strategy (v2):
- Host: add self-loops, sort edges by dst, shard nodes (and their incoming
  edges) across 8 cores in contiguous 2500-node ranges so segment softmax is
  core-local. Within each 128-dst chunk, edges are sorted by src. Channels
  permuted (positive-att first); att folded into the weight columns
  (signed, like table = x @ (W * att)); the Wr-side weights carry an extra
  sum-column (Wr@1) so the linear part of the attention score rides the
  node GEMM and the expansion matmul.
- Device per core: replicated layer-1 GEMM (x replicated); dma_gather of
  256B-aligned table rows per edge (src); xr[dst] expansion as fp8 mask
  matmul PLUS an identity-matmul accumulate of the gathered rows, so
  v = xl[src]+xr[dst] is formed on the TensorEngine in PSUM; Act copies v
  to SBUF bf16 in 2-tile groups; score e = 0.4*(1.5*(gsum+rsum) +
  sum_pos|v| - sum_neg|v|) where the |v| sums and the gathered-row sum come
  from fused DVE tensor_scalar+accum_out ops (4x mode); exp via Act
  (scale=0.4); the dst-scatter mask is generated on-chip fused with the ee
  scale ((iota==dm)*ee) and the aggregation runs as one matmul per tile
  with a ones-column providing the softmax denominator. PSUM->SBUF table
  copies ride the idle Pool engine. Layer 2 is data-parallel over nodes
  with one AllGather of the layer-2 table (quarters, overlapped with
  layer-1 edge compute). No Prelu anywhere so a single activation-table
  set (exp/ln/identity/abs) is loaded once.
"""
import os
import numpy as np
import ml_dtypes

NO_CC = os.environ.get("K_NO_CC", "0") == "1"  # replace collectives (sim only)

N, E0, D, H, A = 20000, 320000, 256, 256, 128
NC = 8
NPC = N // NC            # 2500 real nodes per core
NCHUNK = 20              # chunks of 128 dst nodes
NPCP = NCHUNK * 128      # 2560 padded nodes per core
NN = NC * NPCP           # padded global node rows (20480)
NEG_SLOPE = 0.2
EPS = 1e-30
PSZ = [6, 6, 6, 2]       # layer-2 table piece sizes (chunks)
PST = [0, 6, 12, 18]     # piece starts

_CACHE = {}


def _preprocess(x, edge_index, Wl1, Wr1, att1, b1, Wl2, Wr2, att2, b2):
    bf16 = ml_dtypes.bfloat16
    fp8 = ml_dtypes.float8_e4m3

    perm1 = np.argsort(att1 < 0, kind="stable")
    perm2 = np.argsort(att2 < 0, kind="stable")
    P1p = int((att1 >= 0).sum())
    P2p = int((att2 >= 0).sum())
    a1p = att1[perm1]
    a2p = att2[perm2]

    # signed fold: table entries are att_c * (W x)_c; the score's linear
    # part is then a plain row-sum, and |entry| = |att_c (Wx)_c| for the
    # abs part (signs handled by the pos/neg column split). The Wr-side
    # linear term is constant per dst segment, so it cancels in the
    # softmax and is dropped entirely.
    W1 = np.concatenate([(Wl1 * att1[None, :])[:, perm1],
                         (Wr1 * att1[None, :])[:, perm1]], axis=1).astype(bf16)
    Wl2p = (Wl2 * att2[None, :])[perm1][:, perm2]
    Wr2p = (Wr2 * att2[None, :])[perm1][:, perm2]
    W2 = np.concatenate([Wl2p, Wr2p], axis=1).astype(bf16)

    inv1 = (1.0 / a1p).astype(np.float32).reshape(2, 128).T.copy()  # [128, 2]
    b1c = b1[perm1].astype(np.float32).reshape(2, 128).T.copy()
    inv2_mat = np.tile((1.0 / a2p).astype(np.float32)[None, :], (128, 1))
    b2_mat = np.tile(b2[perm2].astype(np.float32)[None, :], (128, 1))

    # edges + self loops, sharded by dst range, chunked by dst>>7
    loops = np.arange(N, dtype=np.int64)
    src = np.concatenate([edge_index[0].astype(np.int64), loops])
    dst = np.concatenate([edge_index[1].astype(np.int64), loops])
    order = np.argsort(dst, kind="stable")
    src, dst = src[order], dst[order]
    core_of = dst // NPC
    dstl = dst - core_of * NPC
    chunk = dstl >> 7
    counts = np.zeros((NC, NCHUNK), np.int64)
    np.add.at(counts, (core_of, chunk), 1)
    TC = np.ceil(counts.max(axis=0) / 128).astype(np.int64)
    EC = TC * 128
    NI = int(EC.sum())
    T_total = int(TC.sum())

    pad = (src // NPC) * NPCP + (src % NPC)       # padded node id
    # layer1 table rows: partition-major flat [128, NN/128, 256]; layer2
    # table in 4 uneven pieces of [6,6,6,2] chunks (small tail so the last
    # AllGather barely delays the layer-2 edge phase)
    r1 = (pad % 128) * (NN // 128) + pad // 128
    jj = (pad % NPCP) // 128          # chunk index of the src node
    qq = np.minimum(jj // 6, 3)       # piece
    psz = np.array(PSZ)[qq]
    pst = np.array(PST)[qq]
    r2 = pst * (NC * 128) + ((pad // NPCP) * 128 + pad % NPCP % 128) * psz \
        + (jj - pst)
    src_row = r1.astype(np.int32)
    src_row2 = r2.astype(np.int32)

    per_core = []
    for c in range(NC):
        m = core_of == c
        s_c, s2_c, dl_c, ch_c = src_row[m], src_row2[m], dstl[m], chunk[m]
        sr = np.zeros(NI, np.int32)       # layer-1 table rows (pad: 0)
        sr2 = np.zeros(NI, np.int32)      # layer-2 table rows (pad: 0)
        dm = np.full(NI, 999, np.int32)   # dst-local-in-chunk (pad: no match)
        off = 0
        for q in range(NCHUNK):
            sel = ch_c == q
            n = int(sel.sum())
            sq, s2q, dq = s_c[sel], s2_c[sel], dl_c[sel] - q * 128
            o2 = np.argsort(sq, kind="stable")  # src order -> HBM locality
            sr[off:off + n] = sq[o2]
            sr2[off:off + n] = s2q[o2]
            dm[off:off + n] = dq[o2]
            off += int(EC[q])

        def wrap(a):
            out = np.zeros((128, NI // 16), np.int16)
            off_e = 0
            for q in range(NCHUNK):
                e = int(EC[q])
                blk = a[off_e:off_e + e].reshape(e // 16, 16).T.astype(np.int16)
                out[:, off_e // 16:(off_e + e) // 16] = np.tile(blk, (8, 1))
                off_e += e
            return out
        idx_src = wrap(sr)
        idx_src2 = wrap(sr2)
        dmt = dm.reshape(T_total, 128)
        dmf = np.ascontiguousarray(dmt.T).astype(np.float32)  # [128, T_total]
        masks = (dmt[:, :, None] == np.arange(128)[None, None, :])
        masksDE = np.ascontiguousarray(
            masks.transpose(2, 0, 1).reshape(128, T_total * 128)
        ).astype(np.float32).astype(fp8)          # [d-part, (t, e)]

        xoT = np.zeros((D, NPCP), np.float32)
        xoT[:, :NPC] = x[c * NPC:(c + 1) * NPC].T
        per_core.append(dict(
            xoT=xoT.astype(bf16), W1=W1, W2=W2,
            inv1=inv1, b1c=b1c, inv2_mat=inv2_mat, b2_mat=b2_mat,
            idx_src=idx_src, idx_src2=idx_src2, dmf=dmf,
            masksDE=masksDE,
            ident=np.eye(128, dtype=bf16),
            iota=np.tile(np.arange(128, dtype=np.float32)[None, :],
                         (128, 1)).astype(bf16),
        ))

    # replicated padded x^T (same for all cores)
    xf = np.zeros((NN, D), np.float32)
    for c in range(NC):
        xf[c * NPCP:c * NPCP + NPC] = x[c * NPC:(c + 1) * NPC]
    xfT = np.ascontiguousarray(xf.T).astype(bf16)
    for c in range(NC):
        per_core[c]["xfT"] = xfT

    return per_core, [int(t) for t in TC], P1p, P2p, NI, perm2


def _build(TC, P1p, P2p, NI, USE_B2=True):
    from concourse import mybir, tile, bacc

    F32 = mybir.dt.float32
    BF16 = mybir.dt.bfloat16
    FP8 = mybir.dt.float8e4
    I16 = mybir.dt.int16
    AF = mybir.ActivationFunctionType
    OP = mybir.AluOpType
    T_total = sum(TC)
    TCm = max(TC)

    nc = bacc.Bacc("TRN2", target_bir_lowering=False, debug=False,
                   num_devices=NC)
    xfT_d = nc.dram_tensor("xfT", [D, NN], BF16, kind="ExternalInput")
    xoT_d = nc.dram_tensor("xoT", [D, NPCP], BF16, kind="ExternalInput")
    W1_d = nc.dram_tensor("W1", [D, 2 * H], BF16, kind="ExternalInput")
    W2_d = nc.dram_tensor("W2", [H, 2 * A], BF16, kind="ExternalInput")
    inv1_d = nc.dram_tensor("inv1", [128, 2], F32, kind="ExternalInput")
    b1c_d = nc.dram_tensor("b1c", [128, 2], F32, kind="ExternalInput")
    inv2_d = nc.dram_tensor("inv2_mat", [128, A], F32, kind="ExternalInput")
    b2m_d = nc.dram_tensor("b2_mat", [128, A], F32, kind="ExternalInput")
    isrc_d = nc.dram_tensor("idx_src", [128, NI // 16], I16, kind="ExternalInput")
    isrc2_d = nc.dram_tensor("idx_src2", [128, NI // 16], I16, kind="ExternalInput")
    dmf_d = nc.dram_tensor("dmf", [128, T_total], F32, kind="ExternalInput")
    mDE_d = nc.dram_tensor("masksDE", [128, T_total * 128], FP8, kind="ExternalInput")
    iden_d = nc.dram_tensor("ident", [128, 128], BF16, kind="ExternalInput")
    iota_d = nc.dram_tensor("iota", [128, 128], BF16, kind="ExternalInput")
    out_d = nc.dram_tensor("out", [NPCP, A], F32, kind="ExternalOutput")

    with tile.TileContext(nc) as tc:
        with tc.tile_pool(name="const", bufs=1) as cp, \
             tc.tile_pool(name="tabs", bufs=1) as tp, \
             tc.tile_pool(name="edge", bufs=2) as ep, \
             tc.tile_pool(name="small", bufs=2) as sp, \
             tc.tile_pool(name="psg", bufs=2, space="PSUM") as psg, \
             tc.tile_pool(name="psa", bufs=2, space="PSUM") as psa, \
             tc.tile_pool(name="psv", bufs=2, space="PSUM") as psv, \
             tc.tile_pool(name="dram", bufs=1, space="DRAM") as dp:
            # ---- constants / inputs to SBUF
            xoT_sb = cp.tile([128, 2, NPCP], BF16)
            nc.sync.dma_start(out=xoT_sb[:, 0, :], in_=xoT_d[0:128, :])
            nc.sync.dma_start(out=xoT_sb[:, 1, :], in_=xoT_d[128:256, :])
            W1_sb = cp.tile([128, 2, 2 * H], BF16)
            nc.sync.dma_start(out=W1_sb[:, 0, :], in_=W1_d[0:128, :])
            nc.sync.dma_start(out=W1_sb[:, 1, :], in_=W1_d[128:256, :])
            W2_sb = cp.tile([128, 2, 2 * A], BF16)
            nc.sync.dma_start(out=W2_sb[:, 0, :], in_=W2_d[0:128, :])
            nc.sync.dma_start(out=W2_sb[:, 1, :], in_=W2_d[128:256, :])
            inv1_sb = cp.tile([128, 2], F32)
            nc.sync.dma_start(out=inv1_sb[:], in_=inv1_d[:])
            b1c_sb = cp.tile([128, 2], F32)
            nc.sync.dma_start(out=b1c_sb[:], in_=b1c_d[:])
            inv2_sb = cp.tile([128, A], F32)
            nc.sync.dma_start(out=inv2_sb[:], in_=inv2_d[:])
            b2m_sb = cp.tile([128, A], F32)
            nc.sync.dma_start(out=b2m_sb[:], in_=b2m_d[:])
            iden_sb = cp.tile([128, 128], BF16)
            nc.sync.dma_start(out=iden_sb[:], in_=iden_d[:])
            iota_sb = cp.tile([128, 128], BF16)
            nc.sync.dma_start(out=iota_sb[:], in_=iota_d[:])
            ones_sb = cp.tile([128, 1], BF16)
            nc.vector.memset(ones_sb[:], 1.0)
            # the big replicated-x load comes after the GEMM constants so
            # the Wr-side GEMMs can start immediately; edge-phase metadata
            # (indices, dm) loads after it since gathers only start once
            # the table is written anyway
            xfT_sb = cp.tile([128, 2, NN], BF16, tag="xfT")
            for grp in range(NC):
                sl = slice(grp * NPCP, (grp + 1) * NPCP)
                nc.sync.dma_start(out=xfT_sb[:, 0, sl], in_=xfT_d[0:128, sl])
                nc.sync.dma_start(out=xfT_sb[:, 1, sl], in_=xfT_d[128:256, sl])
            isrc_sb = cp.tile([128, NI // 16], I16)
            nc.sync.dma_start(out=isrc_sb[:], in_=isrc_d[:])
            isrc2_sb = cp.tile([128, NI // 16], I16)
            nc.sync.dma_start(out=isrc2_sb[:], in_=isrc2_d[:])
            dmf_sb = cp.tile([128, T_total], F32)
            nc.sync.dma_start(out=dmf_sb[:], in_=dmf_d[:])

            t1full = dp.tile([128, NN // 128, H], BF16)
            t2full = dp.tile([NN, A], BF16)
            t2g = [dp.tile([NC * 128, PSZ[g], A], BF16,
                           **({} if NO_CC else dict(addr_space="Shared")),
                           name=f"t2g_{g}") for g in range(4)]

            # ---- layer 1 tables: replicated GEMM over all nodes (Wl side),
            # own-range GEMM for the Wr side incl. sum-column (stays in
            # SBUF). PSUM->SBUF copies ride the idle Pool engine.
            def any_copy(i, out, in_):
                # round-robin PSUM->SBUF copies across the three idle-ish
                # engines so the table phase isn't gated by one of them
                r = i % 3
                if r == 0:
                    nc.vector.tensor_copy(out=out, in_=in_)
                elif r == 1:
                    nc.scalar.activation(out=out, in_=in_, func=AF.Identity)
                else:
                    nc.gpsimd.tensor_copy(out=out, in_=in_)

            T1r = tp.tile([128, NCHUNK, H], BF16, tag="tr")
            for m in range(NCHUNK):
                ps = psg.tile([128, H], F32, space="PSUM")
                for k in range(2):
                    nc.tensor.matmul(
                        out=ps[:], lhsT=xoT_sb[:, k, m * 128:(m + 1) * 128],
                        rhs=W1_sb[:, k, H:2 * H], start=(k == 0), stop=(k == 1))
                any_copy(m, T1r[:, m, :], ps[:])
            for grp in range(NC):
                grp_sb = tp.tile([128, NCHUNK, H], BF16, tag="town", bufs=2)
                for m in range(NCHUNK):
                    mg = grp * NCHUNK + m
                    ps = psg.tile([128, H], F32, space="PSUM")
                    for k in range(2):
                        nc.tensor.matmul(
                            out=ps[:],
                            lhsT=xfT_sb[:, k, mg * 128:(mg + 1) * 128],
                            rhs=W1_sb[:, k, 0:H], start=(k == 0), stop=(k == 1))
                    any_copy(mg, grp_sb[:, m, :], ps[:])
                nc.sync.dma_start(
                    out=t1full[:, grp * NCHUNK:(grp + 1) * NCHUNK, :],
                    in_=grp_sb[:])

            def edge_layer(CH, Pp, full_dram, idx_sb, r_of, out_cb):
                """CH channels; Pp positive-att channels; r_of(c) gives the
                [128, CH] Wr-side table slice; out_cb(c, out_ps, den_r)
                consumes the chunk PSUM accumulator [128, CH+1] (0:CH
                numer, CH denom)."""
                off_e = 0
                ti0 = 0
                for c in range(NCHUNK):
                    T = TC[c]
                    ECc = T * 128
                    g = ep.tile([128, TCm, CH], BF16, tag="g", bufs=3)
                    src_flat = (full_dram[:] if len(full_dram.shape) == 2
                                else full_dram[:].rearrange("p m c -> (p m) c"))
                    # two half-chunk gathers: finer grain keeps the SWDGE
                    # ring fed and lets the first tiles' matmuls start while
                    # the second half is still gathering
                    Th = (T + 1) // 2
                    for h0, h1 in ((0, Th), (Th, T)):
                        e0, e1 = off_e + h0 * 128, off_e + h1 * 128
                        nc.gpsimd.dma_gather(
                            g[:, h0:h1, :],
                            src_flat,
                            idx_sb[:, e0 // 16:e1 // 16],
                            e1 - e0, e1 - e0, CH, single_packet=False)
                    mkDE = ep.tile([128, TCm * 128], FP8, tag="mkDE")
                    nc.sync.dma_start(
                        out=mkDE[:, 0:ECc],
                        in_=mDE_d[:, ti0 * 128:ti0 * 128 + ECc])
                    # per-edge linear part: plain row-sum of the gathered
                    # (att-folded) rows, via fused tensor_scalar+accum
                    # (per-dst shifts cancel in the softmax, so the Wr side
                    # contributes nothing linear)
                    eA = sp.tile([128, TCm], F32, tag="eA", bufs=3)
                    eB = sp.tile([128, TCm], F32, tag="eB", bufs=3)
                    gs = sp.tile([128, TCm], F32, tag="gs", bufs=3)
                    for t in range(T):
                        zj = sp.tile([128, CH], BF16, tag="zj0")
                        nc.vector.tensor_scalar(
                            out=zj[:], in0=g[:, t, :],
                            scalar1=0.0, scalar2=None, op0=OP.add,
                            accum_out=gs[:, t:t + 1])
                    # v[e,:] = g[e,:] + T_r[dst_e,:]: expansion matmul + an
                    # identity accumulate, grouped 2 tiles per PSUM buffer
                    vsb = sp.tile([128, TCm, CH], BF16, tag="vsb", bufs=2)
                    for t0 in range(0, T, 2):
                        G = min(2, T - t0)
                        vps = psv.tile([128, 2, 256], F32, space="PSUM",
                                       bufs=3)
                        for t in range(t0, t0 + G):
                            nc.tensor.matmul(
                                out=vps[:, t - t0, 0:CH],
                                lhsT=mkDE[:, t * 128:(t + 1) * 128],
                                rhs=r_of(c), start=True, stop=False)
                            nc.tensor.matmul(
                                out=vps[:, t - t0, 0:CH],
                                lhsT=iden_sb[:],
                                rhs=g[:, t, :], start=False, stop=True)
                        nc.scalar.activation(
                            out=vsb[:, t0:t0 + G, :], in_=vps[:, 0:G, 0:CH],
                            func=AF.Identity)
                        for t in range(t0, t0 + G):
                            zj = sp.tile([128, CH - Pp], BF16, tag="zj")
                            nc.vector.tensor_scalar(
                                out=zj[:, 0:Pp], in0=vsb[:, t, 0:Pp],
                                scalar1=0.0, scalar2=None, op0=OP.abs_max,
                                accum_out=eA[:, t:t + 1])
                            nc.vector.tensor_scalar(
                                out=zj[:, 0:CH - Pp], in0=vsb[:, t, Pp:CH],
                                scalar1=0.0, scalar2=None, op0=OP.abs_max,
                                accum_out=eB[:, t:t + 1])
                    # e = 0.4*(1.5*gs + eA - eB); ee = exp(e)
                    d1 = sp.tile([128, TCm], F32, tag="d1", bufs=3)
                    nc.vector.tensor_tensor(
                        out=d1[:, 0:T], in0=eA[:, 0:T], in1=eB[:, 0:T],
                        op=OP.subtract)
                    d15 = sp.tile([128, TCm], F32, tag="d15", bufs=3)
                    nc.vector.tensor_scalar_mul(
                        d15[:, 0:T], gs[:, 0:T], 1.5)
                    d2 = sp.tile([128, TCm], F32, tag="d2", bufs=3)
                    nc.vector.tensor_tensor(
                        out=d2[:, 0:T], in0=d1[:, 0:T], in1=d15[:, 0:T],
                        op=OP.add)
                    ee = sp.tile([128, TCm], F32, tag="ee", bufs=3)
                    nc.scalar.activation(out=ee[:, 0:T], in_=d2[:, 0:T],
                                         func=AF.Exp, scale=0.4)
                    # aggregation: mask rows scaled by ee; a ones-column
                    # matmul accumulates the softmax denominator
                    out_ps = psa.tile([128, CH + 1], F32, space="PSUM")
                    for t in range(T):
                        mk = sp.tile([128, 128], BF16, tag="mk", bufs=4)
                        nc.vector.tensor_scalar(
                            out=mk[:], in0=iota_sb[:],
                            scalar1=dmf_sb[:, ti0 + t:ti0 + t + 1],
                            scalar2=ee[:, t:t + 1],
                            op0=OP.is_equal, op1=OP.mult)
                        nc.tensor.matmul(
                            out=out_ps[:, 0:CH], lhsT=mk[:],
                            rhs=g[:, t, :],
                            start=(t == 0), stop=(t == T - 1))
                        nc.tensor.matmul(
                            out=out_ps[:, CH:CH + 1], lhsT=mk[:],
                            rhs=ones_sb[:],
                            start=(t == 0), stop=(t == T - 1))
                    den = sp.tile([128, 1], F32, tag="den")
                    nc.vector.tensor_scalar(
                        out=den[:], in0=out_ps[:, CH:CH + 1], scalar1=EPS,
                        scalar2=None, op0=OP.add)
                    den_r = sp.tile([128, 1], F32, tag="denr")
                    nc.vector.reciprocal(out=den_r[:], in_=den[:])
                    out_cb(c, out_ps, den_r)
                    off_e += ECc
                    ti0 += T

            # ---- layer 1 edge phase -> h^T (reuses xfT's slot: xfT is dead
            # after the L1 GEMM, and hT [128, 2, NPCP] fits in its slot)
            hT = cp.tile([128, 2, NPCP], BF16, tag="xfT")
            T2both = tp.tile([128, NCHUNK, 2 * A], BF16, tag="town", bufs=2)
            t2own_q = [dp.tile([128, PSZ[g], A], BF16, name=f"t2own_{g}")
                       for g in range(4)]

            def l1_out(c, out_ps, den_r):
                sc = sp.tile([128, H], BF16, tag="sc1")
                nc.scalar.activation(out=sc[:], in_=out_ps[:, 0:H],
                                     func=AF.Identity, scale=den_r[:])
                for b in range(2):
                    tps = psg.tile([128, 128], BF16, space="PSUM", bufs=1)
                    nc.tensor.transpose(
                        out=tps[:], in_=sc[:, b * 128:(b + 1) * 128],
                        identity=iden_sb[:])
                    u = sp.tile([128, 128], BF16, tag="u")
                    nc.scalar.activation(
                        out=u[:], in_=tps[:], func=AF.Identity,
                        scale=inv1_sb[:, b:b + 1], bias=b1c_sb[:, b:b + 1])
                    ng = sp.tile([128, 128], BF16, tag="ng")
                    nc.vector.tensor_scalar_min(ng[:], u[:], 0.0)
                    ex = sp.tile([128, 128], BF16, tag="ex")
                    nc.scalar.activation(out=ex[:], in_=ng[:], func=AF.Exp)
                    px = sp.tile([128, 128], BF16, tag="px")
                    nc.scalar.activation(out=px[:], in_=u[:], func=AF.Relu)
                    nc.vector.scalar_tensor_tensor(
                        out=hT[:, b, c * 128:(c + 1) * 128], in0=ex[:],
                        scalar=-1.0, in1=px[:], op0=OP.add, op1=OP.add)
                ps = psg.tile([128, 2 * A], F32, space="PSUM")
                for k in range(2):
                    nc.tensor.matmul(
                        out=ps[:], lhsT=hT[:, k, c * 128:(c + 1) * 128],
                        rhs=W2_sb[:, k, :], start=(k == 0), stop=(k == 1))
                nc.gpsimd.tensor_copy(out=T2both[:, c, :], in_=ps[:])
                piece_end = {PST[i] + PSZ[i]: i for i in range(4)}
                if (c + 1) in piece_end:
                    g = piece_end[c + 1]
                    nc.sync.dma_start(
                        out=t2own_q[g][:],
                        in_=T2both[:, PST[g]:PST[g] + PSZ[g], 0:A])
                    if NO_CC:
                        for cc in range(NC):
                            nc.sync.dma_start(
                                out=t2g[g][cc * 128:(cc + 1) * 128],
                                in_=t2own_q[g][:])
                    else:
                        nc.gpsimd.collective_compute(
                            "AllGather", mybir.AluOpType.bypass,
                            replica_groups=[list(range(NC))],
                            ins=[t2own_q[g][:].opt()], outs=[t2g[g][:].opt()])
                    r0 = PST[g] * NC * 128
                    r1_ = r0 + PSZ[g] * NC * 128
                    nc.sync.dma_start(
                        out=t2full[r0:r1_, :],
                        in_=t2g[g][:].rearrange("r m c -> (r m) c"))

            edge_layer(H, P1p, t1full, isrc_sb,
                       lambda c: T1r[:, c, :], l1_out)

            # ---- layer 2 edge phase -> log_softmax -> out. The final
            # ln(sum exp) is batched over all chunks at the end so the Act
            # engine never swaps tables (Ln lives in a different act-table
            # set than Exp) during the pipeline.
            out_sb = tp.tile([128, NCHUNK, A], F32, tag="tr")
            rmaxs = tp.tile([128, NCHUNK], F32, tag="rmaxs")
            rsums = tp.tile([128, NCHUNK], F32, tag="rsums")

            def l2_out(c, out_ps, den_r):
                sc = sp.tile([128, A], F32, tag="sc2")
                nc.scalar.activation(out=sc[:], in_=out_ps[:, 0:A],
                                     func=AF.Identity, scale=den_r[:])
                # logits straight into out_sb; exp's accum_out gives the
                # softmax denominator for free (ex2 is scratch)
                if USE_B2:
                    lg = sp.tile([128, A], F32, tag="lg")
                    nc.vector.tensor_tensor(out=lg[:], in0=sc[:],
                                            in1=inv2_sb[:], op=OP.mult)
                    nc.vector.tensor_tensor(out=out_sb[:, c, :], in0=lg[:],
                                            in1=b2m_sb[:], op=OP.add)
                else:
                    nc.vector.tensor_tensor(out=out_sb[:, c, :], in0=sc[:],
                                            in1=inv2_sb[:], op=OP.mult)
                rmax = rmaxs[:, c:c + 1]
                nc.vector.tensor_reduce(
                    out=rmax, in_=out_sb[:, c, :], axis=mybir.AxisListType.X,
                    op=OP.max)
                nrm = sp.tile([128, 1], F32, tag="nrm")
                nc.vector.tensor_scalar_mul(nrm[:], rmax, -1.0)
                ex2 = sp.tile([128, A], F32, tag="ex2")
                nc.scalar.activation(out=ex2[:], in_=out_sb[:, c, :],
                                     func=AF.Exp, bias=nrm[:],
                                     accum_out=rsums[:, c:c + 1])

            edge_layer(A, P2p, t2full, isrc2_sb,
                       lambda c: T2both[:, c, A:2 * A], l2_out)
            lsums = tp.tile([128, NCHUNK], F32, tag="lsums")
            nc.scalar.activation(out=lsums[:], in_=rsums[:], func=AF.Ln)
            shifts = tp.tile([128, NCHUNK], F32, tag="shifts")
            nc.vector.tensor_tensor(out=shifts[:], in0=rmaxs[:],
                                    in1=lsums[:], op=OP.add)
            for c in range(NCHUNK):
                nc.vector.tensor_scalar(
                    out=out_sb[:, c, :], in0=out_sb[:, c, :],
                    scalar1=shifts[:, c:c + 1],
                    scalar2=None, op0=OP.subtract)
            nc.sync.dma_start(
                out=out_d[:].rearrange("(m p) c -> p m c", p=128),
                in_=out_sb[:])

    nc.compile()
    return nc


def kernel(**inputs):
    from concourse.bass_utils import run_bass_kernel_spmd

    per_core, TC, P1p, P2p, NI, perm2 = _preprocess(
        inputs["x"], inputs["edge_index"], inputs["Wl1"], inputs["Wr1"],
        inputs["att1"], inputs["b1"], inputs["Wl2"], inputs["Wr2"],
        inputs["att2"], inputs["b2"])

    use_b2 = bool(np.any(inputs["b2"]))
    key = (tuple(TC), P1p, P2p, NI, use_b2)
    if key not in _CACHE:
        _CACHE[key] = _build(TC, P1p, P2p, NI, use_b2)
    nc = _CACHE[key]

    res = run_bass_kernel_spmd(nc, per_core, list(range(NC)))
    global LAST_RESULT
    LAST_RESULT = res
    out = np.empty((N, A), np.float32)
    for c in range(NC):
        dev = res.results[c]["out"]  # [NPCP, A] in perm2 channel space
        out[c * NPC:(c + 1) * NPC, perm2] = dev[:NPC]
    return out
